# revision 20
# baseline (speedup 1.0000x reference)
# GAT layer kernel for Trainium2 (Bass/Tile), data-parallel over batch:
# one graph per NeuronCore, 8 cores.
#
# Math (per graph, N=2048 nodes, F=128 in, O=64 out):
#   Wh = h @ W + b
#   e[i,j] = leakyrelu(s1[i] + s2[j], 0.2),  s1 = Wh@a1, s2 = Wh@a2
#   att = softmax over i of where(adj>0, e, -inf)
#   out = elu(att^T @ Wh)
#
# Kernel formulation (exact, no approximation):
#   The softmax over i is invariant to per-column-j scaling, so divide
#   exp(leakyrelu(t)) = max(e^t, e^0.2t) by f2[j] = exp(0.2(s2+b.a2)):
#     P[i,j] = adj[i,j] * max(e1[i]*q2[j], f1[i]),
#     e1 = exp(s1+b.a1), f1 = exp(0.2(s1+b.a1)), q2 = exp(0.8(s2+b.a2)).
#   exp is needed only on N-vectors; P is the only N^2 elementwise work.
#   TensorE accumulates accT = [Wh+b|1]^T @ P; row O of accT is the
#   softmax denominator, and b is folded into the stationary (att sums
#   to 1, so (num + b*den)/den = h' + b).  elu via exp(min(x,0)).
#
# Engine layout per iteration:
#   - hT arrives fp16 (host cast); 16 stage-1 matmuls (fp16 stationary,
#     1 cyc/row) pack [Wh|s1] into 3 PSUM banks; 3 batched PSUM->SBUF
#     copies (Act) + per-bank strided exps for e1/f1 read PSUM directly.
#   - q2 broadcast: 4 matmuls with fp16 stationary w2*ones^T stream hl
#     column-chunks; Act fuses exp into the PSUM->SBUF copy, writing
#     natural-j order via a strided dst AP.  No transpose/DMA round-trip.
#   - ones column via one strided memset; b fold via Pool tensor_tensor.
#   - stage 2 (bf16): tensor_scalar (4x mode) + tensor_tensor (2x_1p)
#     per 128-row block; adjacency is int8 in HBM, cast to bf16 by the
#     SWDGE (gpsimd) DMA.  Fp32 path: fused custom DVE op at 1x.
#   - stage 3: one accT copy, transposes packed 7-per-bank, batched
#     copies, ELU via Act relu/exp; processed in two j-halves so the
#     tail pipelines across Act/DVE/DMA.
#
# kernel() accepts the original full inputs; host prep transposes h to
# fp16 hT (virtual-node shuffle), packs adj to int8, and precomputes the
# small derived constants (W3=[W|w1], w2 broadcast, exp biases) that the
# device would otherwise spend serial setup matmuls on.

import numpy as np
import ml_dtypes

import concourse.bacc as bacc
import concourse.mybir as mybir
import concourse.tile as tile
from concourse import masks
from concourse import dve_ops as dvo
from concourse.dve_spec import (
    Spec, Src0, Src1, Zero, C0, C1, maxx, select,
    _has_src1 as has_src1, lower as dve_lower,
)
from concourse.dve_uop import DveOpSpec
from concourse.bass_utils import run_bass_kernel_spmd
from contextlib import ExitStack


def _register_gat_sep():
    """Custom DVE op: P = select(adj != 0, max(in0*s0, s1), 0)."""
    name = "GAT_SEP_MASK_ANT"
    for o in dvo.OPS:
        if o.name == name:
            return o
    body = select(Src1, maxx(Src0 * C0, C1), Zero)

    def _ref(in0, in1, s0, s1, imm2):
        return np.where(in1 != 0,
                        np.maximum(in0.astype(np.float32) * s0, s1),
                        np.float32(0.0)).astype(np.float32)

    spec = Spec(body=body, reference=_ref)
    row = dvo._CUSTOM_DVE_ROW_BASE + len(dvo.OPS)
    assert row < 0x20, "custom DVE opcode rows exhausted"
    shas = {}
    for ver in ("v3", "v4"):
        tmp = DveOpSpec(name=name, opcode=row, uops=dve_lower(spec, ver=ver),
                        rd1_en=has_src1(spec))
        shas[ver] = tmp.sha(ver)
    op = dvo.DveOp(name, spec, subdim=False, uops_sha=shas)
    dvo.OPS.append(op)
    dvo._SUB_OPCODE_FOR_NAME[name] = row
    dvo.CUSTOM_DVE_SPECS[name] = spec
    return op


GAT_SEP = _register_gat_sep()

N = 2048
F = 128
O = 64
B = 8
ALPHA = 0.2

F32 = mybir.dt.float32
F32R = mybir.dt.float32r
FP16 = mybir.dt.float16
BF16 = mybir.dt.bfloat16
I8 = mybir.dt.int8
AF = mybir.ActivationFunctionType
ALU = mybir.AluOpType

RT = N // 128   # 16 row blocks of 128
CW = 512        # matmul chunk width (one PSUM bank of fp32)
CT = N // CW    # 4 chunks
ET = N // 128   # 16 epilogue chunks
RB = 16         # rows per partition in the adjacency DMA layout.  Virtual
                # node v = r*128 + p holds original node p*RB + r; the
                # host shuffles hT columns to match (adjacency unpermuted).

WCOL = O + 1    # [W | w1]; pw cols: Wh, s1
PB = 7          # pw row-blocks packed per PSUM bank (7*65 = 455 <= 512)
NBANK = (RT + PB - 1) // PB      # 3 banks: 7, 7, 2
TPB = 7         # transposes packed per bank in stage 3 (7*65 = 455)

# cst16 layout: [w2_bc (128) | W3 (O+1) | bfold (PB*WCOL) | biases (3)]
C16W = 128 + WCOL + PB * WCOL + 3

# adjacency DMA chunking: small chunks keep supply ahead of the DVE
ADJ_CHUNKS = (1, 1, 2, 2, 2, 2, 2, 2, 2)
assert sum(ADJ_CHUNKS) == RT
# stage-3 transpose grouping (<=7 fits one PSUM bank)
ST3_GROUPS = (7, 7, 2)


def build_gat_kernel(repeat=1, hw_loop=False, dma_only=False, adj_bufs=3,
                     bf16_path=True, pool_cols=0):
    nc = bacc.Bacc("TRN2", target_bir_lowering=False, debug=False, num_devices=B)

    p_dt = BF16 if bf16_path else F32R
    wh_dt = FP16 if bf16_path else F32R
    q2_dt = BF16 if bf16_path else F32
    adj_sb_dt = BF16 if bf16_path else I8

    hT = nc.dram_tensor("hT", [F, N], FP16, kind="ExternalInput").ap()
    adjm = nc.dram_tensor("adjm", [N, N], I8, kind="ExternalInput").ap()
    cst16 = nc.dram_tensor("cst16", [128, C16W], FP16,
                           kind="ExternalInput").ap()
    y = nc.dram_tensor("y", [N, O], F32, kind="ExternalOutput").ap()

    with tile.TileContext(nc) as tc, ExitStack() as ctx:
        const = ctx.enter_context(tc.tile_pool(name="const", bufs=1))
        ld = ctx.enter_context(tc.tile_pool(name="ld", bufs=2))
        ps_blk = ctx.enter_context(tc.tile_pool(name="psblk", bufs=4,
                                                space="PSUM"))
        ps_acc = ctx.enter_context(tc.tile_pool(name="ps_acc", bufs=1,
                                                space="PSUM"))
        adj_pool = ctx.enter_context(tc.tile_pool(name="adjp", bufs=adj_bufs))
        p_pool = ctx.enter_context(tc.tile_pool(name="pp", bufs=2))
        it_pool = ctx.enter_context(tc.tile_pool(name="iter", bufs=2))
        st3_pool = ctx.enter_context(tc.tile_pool(name="st3", bufs=1))

        def blk_tile():
            return ps_blk.tile([128, CW], F32, tag="blk", name="blk")

        # ---------- one-time constants (host-precomputed, 2 DMAs).
        # The dma_starts are deferred into the first body so the hl
        # halves hit the DMA queue first (single-shot prologue).
        c16_sb = const.tile([128, C16W], FP16)
        _consts_loaded = []

        def _load_consts_pre():
            if not _consts_loaded:
                nc.sync.dma_start(c16_sb[:], cst16)
                _consts_loaded.append(True)

        def _load_consts():
            pass
        BIA = 128 + WCOL + PB * WCOL
        b1_bc = c16_sb[:, BIA:BIA + 1]
        b1f_bc = c16_sb[:, BIA + 1:BIA + 2]
        b2f8_bc = c16_sb[:, BIA + 2:BIA + 3]
        w2_bc = c16_sb[:, :128]
        W3_sb = c16_sb[:, 128:128 + WCOL]
        bf_sb = c16_sb[:, 128 + WCOL:]
        # e0: row-0-ones stationary; e0^T @ bf adds [b|0] to every
        # partition of the pw accumulation group (the b fold)
        e0_sb = const.tile([128, 128], FP16)
        nc.vector.memset(e0_sb[:], 0.0)
        nc.vector.memset(e0_sb[:1, :], 1.0)

        ident = const.tile([128, 128], F32)
        masks.make_identity(nc, ident[:])

        ones_row = const.tile([1, 1], F32)
        nc.vector.memset(ones_row[:], 1.0)
        if repeat == 1:
            # PE p-state warmup: keep the tensor engine streaming while the
            # input DMAs land so stage-1/q2 matmuls run at full clock.
            for _w in range(10):
                pwu = blk_tile()
                nc.tensor.transpose(pwu[:, :128], ident[:], ident[:])
        # Warm the Exp/Relu activation tables so the table load overlaps
        # the first DMAs instead of stalling the first exp.
        warm = const.tile([1, 2], F32)
        nc.scalar.activation(warm[:, :1], ones_row[:], AF.Exp)
        nc.scalar.activation(warm[:, 1:], ones_row[:], AF.Relu)

        def _stage1():
            """Issue stage-1 for one iteration; returns its tile state."""
            hl = ld.tile([128, N], FP16, tag="hT")
            HH = N // 2
            _load_consts_pre()
            nc.sync.dma_start(hl[:, :HH], hT[:, :HH])
            nc.sync.dma_start(hl[:, HH:], hT[:, HH:])
            _load_consts()

            # 16 packed stage-1 matmuls -> 3 PSUM banks.  PE order: bank A
            # (needs only the first hl half), then q2 (needs full hl and
            # gates stage 2 via the Act exp chain), then banks B/C.
            # The b fold rides the accumulation group: one extra matmul per
            # bank adds e0^T @ [b|0]-pattern to every partition.
            pwb = [blk_tile() for _ in range(NBANK)]

            def _pw_mms(bank):
                nblk = min(PB, RT - bank * PB)
                for k in range(nblk):
                    r = bank * PB + k
                    rsl = slice(r * 128, (r + 1) * 128)
                    nc.tensor.matmul(pwb[bank][:, k * WCOL:(k + 1) * WCOL],
                                     hl[:, rsl], W3_sb, start=(k == 0),
                                     stop=False)
                nc.tensor.matmul(pwb[bank][:, :nblk * WCOL], e0_sb[:],
                                 bf_sb[:, :nblk * WCOL], start=False,
                                 stop=True)

            whm_all = it_pool.tile([128, RT, O + 1], wh_dt, tag="whm")
            e1_all = it_pool.tile([128, RT], F32, tag="e1a")
            f1_all = it_pool.tile([128, RT], F32, tag="f1a")
            # ones column for the denominator row (independent of copies)
            nc.vector.memset(whm_all[:, :, O:O + 1], 1.0)

            def _bank_view(bank):
                nblk = min(PB, RT - bank * PB)
                bsl = slice(bank * PB, bank * PB + nblk)
                return bsl, pwb[bank][:, :nblk * WCOL].rearrange(
                    "p (k c) -> p k c", c=WCOL)

            def _e1f1(bank):
                bsl, srcv = _bank_view(bank)
                nc.scalar.activation(
                    e1_all[:, bsl], srcv[:, :, O:O + 1], AF.Exp,
                    bias=b1_bc, scale=1.0)
                nc.scalar.activation(
                    f1_all[:, bsl], srcv[:, :, O:O + 1], AF.Exp,
                    bias=b1f_bc, scale=ALPHA)

            def _whm_copy(bank):
                bsl, srcv = _bank_view(bank)
                nc.scalar.copy(whm_all[:, bsl, :O], srcv[:, :, :O])

            # q2 broadcast: out[p, v] = s2[virtual v] for every partition p;
            # the fused exp-copy scatters to natural-j order via a strided
            # dst AP (natural j = p*RB + r for virtual v = r*128 + p).
            q2_bc = it_pool.tile([128, N], q2_dt, tag="q2bc")
            q2_nat = q2_bc[:].rearrange("q (p r) -> q r p", r=RB)
            RPC = CW // 128     # virtual r-blocks per chunk

            def _q2(c):
                pq = blk_tile()
                nc.tensor.matmul(pq[:], w2_bc, hl[:, c * CW:(c + 1) * CW],
                                 start=True, stop=True)
                HC = CW // 2
                for hh in range(2):
                    nc.scalar.activation(
                        q2_nat[:, c * RPC + hh * RPC // 2:
                               c * RPC + (hh + 1) * RPC // 2, :],
                        pq[:, hh * HC:(hh + 1) * HC], AF.Exp,
                        bias=b2f8_bc, scale=1.0 - ALPHA)

            _pw_mms(0)
            _e1f1(0)
            _q2(0)          # chunks 0/1 need only the first hl half
            _q2(1)
            _q2(2)
            _q2(3)
            for bank in range(1, NBANK):
                _pw_mms(bank)
                _e1f1(bank)
            for bank in range(NBANK):
                _whm_copy(bank)
            return dict(whm=whm_all, q2_bc=q2_bc, e1=e1_all, f1=f1_all)

        adj_blk = adjm.rearrange("(p rb) n -> p rb n", rb=RB, p=128)
        _adj_stubbed = []

        def _adj_dma(blk0, nblk):
            adj_t = adj_pool.tile([128, max(ADJ_CHUNKS), N], adj_sb_dt,
                                  tag="adj")
            if blk0 == 0 and not _adj_stubbed:
                # WAR stub: delays chunk 0's SWDGE descriptor until the
                # consts DMA lands so the hT halves win the DMA queue.
                nc.gpsimd.tensor_copy(adj_t[:1, :1, :1], c16_sb[:1, :1])
                _adj_stubbed.append(True)
            if bf16_path:
                nc.gpsimd.dma_start(adj_t[:, :nblk, :],
                                    adj_blk[:, blk0:blk0 + nblk, :])
            else:
                nc.scalar.dma_start(adj_t[:, :nblk, :],
                                    adj_blk[:, blk0:blk0 + nblk, :])
            return adj_t

        def _stage2_blocks(stg, adj_t, blocks):
            q2_bc, whm = stg["q2_bc"], stg["whm"]
            for idx, r in enumerate(blocks):
                p = p_pool.tile([128, N], p_dt, tag="p")
                if bf16_path:
                    pu = p_pool.tile([128, N], BF16, tag="pu")
                    nc.vector.tensor_scalar(pu[:], q2_bc[:],
                                            stg["e1"][:, r:r + 1],
                                            stg["f1"][:, r:r + 1],
                                            op0=ALU.mult, op1=ALU.max)
                    nc.vector.tensor_tensor(p[:], pu[:], adj_t[:, idx, :],
                                            op=ALU.mult)
                else:
                    nc.vector._custom_dve(
                        GAT_SEP, out=p[:], in0=q2_bc[:],
                        in1=adj_t[:, idx, :], s0=stg["e1"][:, r:r + 1],
                        s1=stg["f1"][:, r:r + 1])
                for c in range(CT):
                    csl = slice(c * CW, (c + 1) * CW)
                    nc.tensor.matmul(acc_tiles[c], whm[:, r, :], p[:, csl],
                                     start=(r == 0), stop=(r == RT - 1))

        def _stage3():
            accT = st3_pool.tile([O + 1, N], F32, tag="accT")
            accT_v = accT[:].rearrange("o (c w) -> o c w", w=CW)
            # split the PSUM->SBUF drain across Act and DVE (both idle here)
            nc.vector.tensor_copy(accT_v[:, :2, :], acc_all[:O + 1, :2, :])
            nc.scalar.copy(accT_v[:, 2:, :], acc_all[:O + 1, 2:, :])
            rec_all = st3_pool.tile([128, ET], F32, tag="recall")
            hpall = st3_pool.tile([128, ET, O], F32, tag="hpall")
            t1 = st3_pool.tile([128, ET, O], F32, tag="t1")
            gall = st3_pool.tile([128, ET, O], F32, tag="gall")
            out_sb = st3_pool.tile([128, ET, O], F32, tag="outsb")
            y_v = y.rearrange("(p j) o -> p j o", p=128)
            g0 = 0
            for nblk in ST3_GROUPS:
                gsl = slice(g0, g0 + nblk)
                pt = blk_tile()
                for k in range(nblk):
                    nc.tensor.transpose(
                        pt[:, k * (O + 1):(k + 1) * (O + 1)],
                        accT[:, g0 + k::ET], ident[:O + 1, :O + 1])

                # DVE reads the transposed bank straight from PSUM
                ptv = pt[:, :nblk * (O + 1)].rearrange("p (k c) -> p k c",
                                                       c=O + 1)
                nc.vector.reciprocal(rec_all[:, gsl], ptv[:, :, O])
                rec_bc = rec_all[:, gsl].rearrange("p (r o) -> p r o", o=1)
                nc.vector.tensor_tensor(hpall[:, gsl, :], ptv[:, :, :O],
                                        rec_bc.broadcast_to([128, nblk, O]),
                                        op=ALU.mult)
                # elu: out = max(hp, exp(min(hp,0)) - 1); min stays on DVE
                # (no engine hop after the TT), exp on Act.
                nc.vector.tensor_scalar_min(t1[:, gsl, :], hpall[:, gsl, :],
                                            0.0)
                nc.scalar.activation(gall[:, gsl, :], t1[:, gsl, :],
                                     AF.Exp)
                nc.vector.scalar_tensor_tensor(out_sb[:, gsl, :],
                                               gall[:, gsl, :], -1.0,
                                               hpall[:, gsl, :],
                                               op0=ALU.add, op1=ALU.max)
                nc.scalar.dma_start(y_v[:, gsl, :], out_sb[:, gsl, :])
                g0 += nblk

        if dma_only:
            def _body(_iv=None):
                out_sb = st3_pool.tile([128, ET, O], F32, tag="outsb")
                hl = ld.tile([128, N], FP16, tag="hT")
                HH = N // 2
                _load_consts_pre()
                nc.sync.dma_start(hl[:, :HH], hT[:, :HH])
                nc.sync.dma_start(hl[:, HH:], hT[:, HH:])
                _load_consts()
                blk0 = 0
                for nblk in ADJ_CHUNKS:
                    _adj_dma(blk0, nblk)
                    blk0 += nblk
                nc.vector.memset(out_sb[:], 0.0)
                nc.sync.dma_start(y.rearrange("(p j) o -> p j o", p=128),
                                  out_sb[:])
        else:
            acc_all = ps_acc.tile([O + 1, CT, CW], F32, tag="acc")
            acc_tiles = [acc_all[:, c, :] for c in range(CT)]

            def _body(_iv=None):
                stg = _stage1()
                blk0 = 0
                for ci, nblk in enumerate(ADJ_CHUNKS):
                    adj_t = _adj_dma(blk0, nblk)
                    _stage2_blocks(stg, adj_t, range(blk0, blk0 + nblk))
                    blk0 += nblk
                _stage3()

        if hw_loop and repeat > 1:
            tc.For_i_unrolled(0, repeat, 1, _body, max_unroll=8)
        else:
            for _it in range(repeat):
                _body()

    nc.compile()
    return nc


_NC_CACHE = None


def prep_inputs(h, adj, W, b, a):
    """Host-side re-layout + derived small constants."""
    h = np.ascontiguousarray(h, dtype=np.float32)
    W = np.ascontiguousarray(W, dtype=np.float32)
    b = np.ascontiguousarray(b, dtype=np.float32).reshape(O)
    a = np.ascontiguousarray(a, dtype=np.float32)
    hT = np.swapaxes(h, -1, -2)
    # virtual node v = r*128 + p holds original node p*RB + r
    idx = (np.arange(128)[None, :] * RB + np.arange(RT)[:, None]).reshape(-1)
    hT = np.ascontiguousarray(hT[..., idx]).astype(np.float16)
    adjm = (np.asarray(adj) != 0).astype(np.int8)

    a1, a2 = a[:O, 0], a[O:, 0]
    w1 = W @ a1                       # [F]
    w2 = W @ a2                       # [F]
    beta1 = float(b @ a1)
    beta2 = float(b @ a2)
    cst16 = np.zeros((128, C16W), np.float16)
    cst16[:, -3] = beta1
    cst16[:, -2] = ALPHA * beta1
    cst16[:, -1] = (1.0 - ALPHA) * beta2
    cst16[:, :128] = w2[:, None]      # w2 broadcast along free dim
    cst16[:, 128:128 + O] = W
    cst16[:, 128 + O] = w1
    bf = np.concatenate([b, [0.0]]).astype(np.float16)   # [b | 0] per block
    cst16[0, 128 + WCOL:128 + WCOL + PB * WCOL] = np.tile(bf, PB)
    # only row 0 is read by the e0 stationary; zero elsewhere already
    return hT, adjm, cst16


def make_in_maps(h, adj, W, b, a):
    hT, adjm, cst16 = prep_inputs(h, adj, W, b, a)
    return [
        {"hT": hT[i], "adjm": adjm[i], "cst16": cst16}
        for i in range(B)
    ]


def kernel(h, adj, W, b, a):
    global _NC_CACHE
    in_maps = make_in_maps(h, adj, W, b, a)
    if _NC_CACHE is None:
        _NC_CACHE = build_gat_kernel()
    nc = _NC_CACHE
    res = run_bass_kernel_spmd(nc, in_maps, core_ids=list(range(B)))
    out = np.stack([r["y"] for r in res.results], axis=0)
    return out


# revision 21
# speedup vs baseline: 2.3189x; 2.3189x over previous
# GAT layer kernel for Trainium2 (Bass/Tile), data-parallel over batch:
# one graph per NeuronCore, 8 cores.
#
# Math (per graph, N=2048 nodes, F=128 in, O=64 out):
#   Wh = h @ W + b
#   e[i,j] = leakyrelu(s1[i] + s2[j], 0.2),  s1 = Wh@a1, s2 = Wh@a2
#   att = softmax over i of where(adj>0, e, -inf)
#   out = elu(att^T @ Wh)
#
# Kernel formulation (exact, no approximation):
#   The softmax over i is invariant to per-column-j scaling, so divide
#   exp(leakyrelu(t)) = max(e^t, e^0.2t) by f2[j] = exp(0.2(s2+b.a2)):
#     P[i,j] = adj[i,j] * max(e1[i]*q2[j], f1[i]),
#     e1 = exp(s1+b.a1), f1 = exp(0.2(s1+b.a1)), q2 = exp(0.8(s2+b.a2)).
#   exp is needed only on N-vectors; P is the only N^2 elementwise work.
#   TensorE accumulates accT = [Wh+b|1]^T @ P; row O of accT is the
#   softmax denominator, and b is folded into the stationary (att sums
#   to 1, so (num + b*den)/den = h' + b).  elu via exp(min(x,0)).
#
# Engine layout per iteration:
#   - hT arrives fp16 (host cast); 16 stage-1 matmuls (fp16 stationary,
#     1 cyc/row) pack [Wh|s1] into 3 PSUM banks; 3 batched PSUM->SBUF
#     copies (Act) + per-bank strided exps for e1/f1 read PSUM directly.
#   - q2 broadcast: 4 matmuls with fp16 stationary w2*ones^T stream hl
#     column-chunks; Act fuses exp into the PSUM->SBUF copy, writing
#     natural-j order via a strided dst AP.  No transpose/DMA round-trip.
#   - ones column via one strided memset; b fold via Pool tensor_tensor.
#   - stage 2 (bf16): tensor_scalar (4x mode) + tensor_tensor (2x_1p)
#     per 128-row block; adjacency is int8 in HBM, cast to bf16 by the
#     SWDGE (gpsimd) DMA.  Fp32 path: fused custom DVE op at 1x.
#   - stage 3: one accT copy, transposes packed 7-per-bank, batched
#     copies, ELU via Act relu/exp; processed in two j-halves so the
#     tail pipelines across Act/DVE/DMA.
#
# kernel() accepts the original full inputs; host prep transposes h to
# fp16 hT (virtual-node shuffle), packs adj to int8, and precomputes the
# small derived constants (W3=[W|w1], w2 broadcast, exp biases) that the
# device would otherwise spend serial setup matmuls on.

import numpy as np
import ml_dtypes

import concourse.bacc as bacc
import concourse.mybir as mybir
import concourse.tile as tile
from concourse import masks
from concourse import dve_ops as dvo
from concourse.dve_spec import (
    Spec, Src0, Src1, Zero, C0, C1, maxx, select,
    _has_src1 as has_src1, lower as dve_lower,
)
from concourse.dve_uop import DveOpSpec
from concourse.bass_utils import run_bass_kernel_spmd
from contextlib import ExitStack


def _register_gat_sep():
    """Custom DVE op: P = select(adj != 0, max(in0*s0, s1), 0)."""
    name = "GAT_SEP_MASK_ANT"
    for o in dvo.OPS:
        if o.name == name:
            return o
    body = select(Src1, maxx(Src0 * C0, C1), Zero)

    def _ref(in0, in1, s0, s1, imm2):
        return np.where(in1 != 0,
                        np.maximum(in0.astype(np.float32) * s0, s1),
                        np.float32(0.0)).astype(np.float32)

    spec = Spec(body=body, reference=_ref)
    row = dvo._CUSTOM_DVE_ROW_BASE + len(dvo.OPS)
    assert row < 0x20, "custom DVE opcode rows exhausted"
    shas = {}
    for ver in ("v3", "v4"):
        tmp = DveOpSpec(name=name, opcode=row, uops=dve_lower(spec, ver=ver),
                        rd1_en=has_src1(spec))
        shas[ver] = tmp.sha(ver)
    op = dvo.DveOp(name, spec, subdim=False, uops_sha=shas)
    dvo.OPS.append(op)
    dvo._SUB_OPCODE_FOR_NAME[name] = row
    dvo.CUSTOM_DVE_SPECS[name] = spec
    return op


GAT_SEP = _register_gat_sep()

N = 2048
F = 128
O = 64
B = 8
ALPHA = 0.2

F32 = mybir.dt.float32
F32R = mybir.dt.float32r
FP16 = mybir.dt.float16
BF16 = mybir.dt.bfloat16
I8 = mybir.dt.int8
AF = mybir.ActivationFunctionType
ALU = mybir.AluOpType

RT = N // 128   # 16 row blocks of 128
CW = 512        # matmul chunk width (one PSUM bank of fp32)
CT = N // CW    # 4 chunks
ET = N // 128   # 16 epilogue chunks
RB = 16         # rows per partition in the adjacency DMA layout.  Virtual
                # node v = r*128 + p holds original node p*RB + r; the
                # host shuffles hT columns to match (adjacency unpermuted).

WCOL = O + 1    # [W | w1]; pw cols: Wh, s1
PB = 7          # pw row-blocks packed per PSUM bank (7*65 = 455 <= 512)
NBANK = (RT + PB - 1) // PB      # 3 banks: 7, 7, 2
TPB = 7         # transposes packed per bank in stage 3 (7*65 = 455)

# cst16 layout: [w2_bc (128) | W3 (O+1) | bfold (PB*WCOL) | biases (3)]
C16W = 128 + WCOL + PB * WCOL + 3

# adjacency DMA chunking: small chunks keep supply ahead of the DVE
ADJ_CHUNKS = (1, 1, 2, 2, 2, 2, 2, 2, 2)
assert sum(ADJ_CHUNKS) == RT
# stage-3 transpose grouping (<=7 fits one PSUM bank)
ST3_GROUPS = (7, 7, 2)


ADJ_HOST_BF16 = False   # host sends bf16 adjacency (HWDGE) instead of
                        # int8 + SWDGE cast-DMA


def build_gat_kernel(repeat=1, hw_loop=False, dma_only=False, adj_bufs=3,
                     bf16_path=True, pool_cols=0, adj_host_bf16=None):
    nc = bacc.Bacc("TRN2", target_bir_lowering=False, debug=False, num_devices=B)

    if adj_host_bf16 is None:
        adj_host_bf16 = ADJ_HOST_BF16 and bf16_path
    p_dt = BF16 if bf16_path else F32R
    wh_dt = FP16 if bf16_path else F32R
    q2_dt = BF16 if bf16_path else F32
    adj_sb_dt = BF16 if bf16_path else I8

    hT = nc.dram_tensor("hT", [F, N], FP16, kind="ExternalInput").ap()
    adjm = nc.dram_tensor("adjm", [N, N],
                          BF16 if adj_host_bf16 else I8,
                          kind="ExternalInput").ap()
    cst16 = nc.dram_tensor("cst16", [128, C16W], FP16,
                           kind="ExternalInput").ap()
    y = nc.dram_tensor("y", [N, O], F32, kind="ExternalOutput").ap()

    with tile.TileContext(nc) as tc, ExitStack() as ctx:
        const = ctx.enter_context(tc.tile_pool(name="const", bufs=1))
        ld = ctx.enter_context(tc.tile_pool(name="ld", bufs=2))
        ps_blk = ctx.enter_context(tc.tile_pool(name="psblk", bufs=4,
                                                space="PSUM"))
        ps_acc = ctx.enter_context(tc.tile_pool(name="ps_acc", bufs=1,
                                                space="PSUM"))
        adj_pool = ctx.enter_context(tc.tile_pool(name="adjp", bufs=adj_bufs))
        p_pool = ctx.enter_context(tc.tile_pool(name="pp", bufs=2))
        it_pool = ctx.enter_context(tc.tile_pool(name="iter", bufs=2))
        st3_pool = ctx.enter_context(tc.tile_pool(name="st3", bufs=1))

        def blk_tile():
            return ps_blk.tile([128, CW], F32, tag="blk", name="blk")

        # ---------- one-time constants (host-precomputed, 2 DMAs).
        # The dma_starts are deferred into the first body so the hl
        # halves hit the DMA queue first (single-shot prologue).
        c16_sb = const.tile([128, C16W], FP16)
        _consts_loaded = []

        def _load_consts_pre():
            if not _consts_loaded:
                nc.sync.dma_start(c16_sb[:], cst16)
                _consts_loaded.append(True)

        def _load_consts():
            pass
        BIA = 128 + WCOL + PB * WCOL
        b1_bc = c16_sb[:, BIA:BIA + 1]
        b1f_bc = c16_sb[:, BIA + 1:BIA + 2]
        b2f8_bc = c16_sb[:, BIA + 2:BIA + 3]
        w2_bc = c16_sb[:, :128]
        W3_sb = c16_sb[:, 128:128 + WCOL]
        bf_sb = c16_sb[:, 128 + WCOL:]
        # e0: row-0-ones stationary; e0^T @ bf adds [b|0] to every
        # partition of the pw accumulation group (the b fold)
        e0_sb = const.tile([128, 128], FP16)
        nc.vector.memset(e0_sb[:], 0.0)
        nc.vector.memset(e0_sb[:1, :], 1.0)

        ident = const.tile([128, 128], F32)
        masks.make_identity(nc, ident[:])

        ones_row = const.tile([1, 1], F32)
        nc.vector.memset(ones_row[:], 1.0)
        if repeat == 1:
            # PE p-state warmup: keep the tensor engine streaming while the
            # input DMAs land so stage-1/q2 matmuls run at full clock.
            for _w in range(10):
                pwu = blk_tile()
                nc.tensor.transpose(pwu[:, :128], ident[:], ident[:])
        # Warm the Exp/Relu activation tables so the table load overlaps
        # the first DMAs instead of stalling the first exp.
        warm = const.tile([1, 2], F32)
        nc.scalar.activation(warm[:, :1], ones_row[:], AF.Exp)
        nc.scalar.activation(warm[:, 1:], ones_row[:], AF.Relu)

        def _stage1():
            """Issue stage-1 for one iteration; returns its tile state."""
            hl = ld.tile([128, N], FP16, tag="hT")
            HH = N // 2
            _load_consts_pre()
            nc.sync.dma_start(hl[:, :HH], hT[:, :HH])
            nc.sync.dma_start(hl[:, HH:], hT[:, HH:])
            _load_consts()

            # 16 packed stage-1 matmuls -> 3 PSUM banks.  PE order: bank A
            # (needs only the first hl half), then q2 (needs full hl and
            # gates stage 2 via the Act exp chain), then banks B/C.
            # The b fold rides the accumulation group: one extra matmul per
            # bank adds e0^T @ [b|0]-pattern to every partition.
            pwb = [blk_tile() for _ in range(NBANK)]

            def _pw_mms(bank):
                nblk = min(PB, RT - bank * PB)
                for k in range(nblk):
                    r = bank * PB + k
                    rsl = slice(r * 128, (r + 1) * 128)
                    nc.tensor.matmul(pwb[bank][:, k * WCOL:(k + 1) * WCOL],
                                     hl[:, rsl], W3_sb, start=(k == 0),
                                     stop=False)
                nc.tensor.matmul(pwb[bank][:, :nblk * WCOL], e0_sb[:],
                                 bf_sb[:, :nblk * WCOL], start=False,
                                 stop=True)

            whm_all = it_pool.tile([128, RT, O + 1], wh_dt, tag="whm")
            e1_all = it_pool.tile([128, RT], F32, tag="e1a")
            f1_all = it_pool.tile([128, RT], F32, tag="f1a")
            # ones column for the denominator row (independent of copies)
            nc.vector.memset(whm_all[:, :, O:O + 1], 1.0)

            def _bank_view(bank):
                nblk = min(PB, RT - bank * PB)
                bsl = slice(bank * PB, bank * PB + nblk)
                return bsl, pwb[bank][:, :nblk * WCOL].rearrange(
                    "p (k c) -> p k c", c=WCOL)

            def _e1f1(bank):
                bsl, srcv = _bank_view(bank)
                nc.scalar.activation(
                    e1_all[:, bsl], srcv[:, :, O:O + 1], AF.Exp,
                    bias=b1_bc, scale=1.0)
                nc.scalar.activation(
                    f1_all[:, bsl], srcv[:, :, O:O + 1], AF.Exp,
                    bias=b1f_bc, scale=ALPHA)

            def _whm_copy(bank):
                bsl, srcv = _bank_view(bank)
                nc.scalar.copy(whm_all[:, bsl, :O], srcv[:, :, :O])

            # q2 broadcast: out[p, v] = s2[virtual v] for every partition p;
            # the fused exp-copy scatters to natural-j order via a strided
            # dst AP (natural j = p*RB + r for virtual v = r*128 + p).
            q2_bc = it_pool.tile([128, N], q2_dt, tag="q2bc")
            q2_nat = q2_bc[:].rearrange("q (p r) -> q r p", r=RB)
            RPC = CW // 128     # virtual r-blocks per chunk

            def _q2(c):
                pq = blk_tile()
                nc.tensor.matmul(pq[:], w2_bc, hl[:, c * CW:(c + 1) * CW],
                                 start=True, stop=True)
                HC = CW // 2
                for hh in range(2):
                    nc.scalar.activation(
                        q2_nat[:, c * RPC + hh * RPC // 2:
                               c * RPC + (hh + 1) * RPC // 2, :],
                        pq[:, hh * HC:(hh + 1) * HC], AF.Exp,
                        bias=b2f8_bc, scale=1.0 - ALPHA)

            _pw_mms(0)
            _e1f1(0)
            _q2(0)          # chunks 0/1 need only the first hl half
            _q2(1)
            _q2(2)
            _q2(3)
            for bank in range(1, NBANK):
                _pw_mms(bank)
                _e1f1(bank)
            for bank in range(NBANK):
                _whm_copy(bank)
            return dict(whm=whm_all, q2_bc=q2_bc, e1=e1_all, f1=f1_all)

        adj_blk = adjm.rearrange("(p rb) n -> p rb n", rb=RB, p=128)
        _adj_stubbed = []

        def _adj_dma(blk0, nblk):
            adj_t = adj_pool.tile([128, max(ADJ_CHUNKS), N], adj_sb_dt,
                                  tag="adj")
            if bf16_path and not adj_host_bf16:
                if blk0 == 0 and not _adj_stubbed:
                    # WAR stub: delays chunk 0's SWDGE descriptor until the
                    # consts DMA lands so the hT halves win the DMA queue.
                    nc.gpsimd.tensor_copy(adj_t[:1, :1, :1], c16_sb[:1, :1])
                    _adj_stubbed.append(True)
                nc.gpsimd.dma_start(adj_t[:, :nblk, :],
                                    adj_blk[:, blk0:blk0 + nblk, :])
            else:
                nc.scalar.dma_start(adj_t[:, :nblk, :],
                                    adj_blk[:, blk0:blk0 + nblk, :])
            return adj_t

        def _stage2_blocks(stg, adj_t, blocks):
            q2_bc, whm = stg["q2_bc"], stg["whm"]
            for idx, r in enumerate(blocks):
                p = p_pool.tile([128, N], p_dt, tag="p")
                if bf16_path:
                    pu = p_pool.tile([128, N], BF16, tag="pu")
                    nc.vector.tensor_scalar(pu[:], q2_bc[:],
                                            stg["e1"][:, r:r + 1],
                                            stg["f1"][:, r:r + 1],
                                            op0=ALU.mult, op1=ALU.max)
                    nc.vector.tensor_tensor(p[:], pu[:], adj_t[:, idx, :],
                                            op=ALU.mult)
                else:
                    nc.vector._custom_dve(
                        GAT_SEP, out=p[:], in0=q2_bc[:],
                        in1=adj_t[:, idx, :], s0=stg["e1"][:, r:r + 1],
                        s1=stg["f1"][:, r:r + 1])
                for c in range(CT):
                    csl = slice(c * CW, (c + 1) * CW)
                    nc.tensor.matmul(acc_tiles[c], whm[:, r, :], p[:, csl],
                                     start=(r == 0), stop=(r == RT - 1))

        def _stage3():
            accT = st3_pool.tile([O + 1, N], F32, tag="accT")
            accT_v = accT[:].rearrange("o (c w) -> o c w", w=CW)
            # split the PSUM->SBUF drain across Act and DVE (both idle here)
            nc.vector.tensor_copy(accT_v[:, :2, :], acc_all[:O + 1, :2, :])
            nc.scalar.copy(accT_v[:, 2:, :], acc_all[:O + 1, 2:, :])
            rec_all = st3_pool.tile([128, ET], F32, tag="recall")
            hpall = st3_pool.tile([128, ET, O], F32, tag="hpall")
            t1 = st3_pool.tile([128, ET, O], F32, tag="t1")
            gall = st3_pool.tile([128, ET, O], F32, tag="gall")
            out_sb = st3_pool.tile([128, ET, O], F32, tag="outsb")
            y_v = y.rearrange("(p j) o -> p j o", p=128)
            g0 = 0
            for nblk in ST3_GROUPS:
                gsl = slice(g0, g0 + nblk)
                pt = blk_tile()
                for k in range(nblk):
                    nc.tensor.transpose(
                        pt[:, k * (O + 1):(k + 1) * (O + 1)],
                        accT[:, g0 + k::ET], ident[:O + 1, :O + 1])

                # DVE reads the transposed bank straight from PSUM
                ptv = pt[:, :nblk * (O + 1)].rearrange("p (k c) -> p k c",
                                                       c=O + 1)
                nc.vector.reciprocal(rec_all[:, gsl], ptv[:, :, O])
                rec_bc = rec_all[:, gsl].rearrange("p (r o) -> p r o", o=1)
                nc.vector.tensor_tensor(hpall[:, gsl, :], ptv[:, :, :O],
                                        rec_bc.broadcast_to([128, nblk, O]),
                                        op=ALU.mult)
                # elu: out = max(hp, exp(min(hp,0)) - 1); min stays on DVE
                # (no engine hop after the TT), exp on Act.
                nc.vector.tensor_scalar_min(t1[:, gsl, :], hpall[:, gsl, :],
                                            0.0)
                nc.scalar.activation(gall[:, gsl, :], t1[:, gsl, :],
                                     AF.Exp)
                nc.vector.scalar_tensor_tensor(out_sb[:, gsl, :],
                                               gall[:, gsl, :], -1.0,
                                               hpall[:, gsl, :],
                                               op0=ALU.add, op1=ALU.max)
                nc.scalar.dma_start(y_v[:, gsl, :], out_sb[:, gsl, :])
                g0 += nblk

        if dma_only:
            def _body(_iv=None):
                out_sb = st3_pool.tile([128, ET, O], F32, tag="outsb")
                hl = ld.tile([128, N], FP16, tag="hT")
                HH = N // 2
                _load_consts_pre()
                nc.sync.dma_start(hl[:, :HH], hT[:, :HH])
                nc.sync.dma_start(hl[:, HH:], hT[:, HH:])
                _load_consts()
                blk0 = 0
                for nblk in ADJ_CHUNKS:
                    _adj_dma(blk0, nblk)
                    blk0 += nblk
                nc.vector.memset(out_sb[:], 0.0)
                nc.sync.dma_start(y.rearrange("(p j) o -> p j o", p=128),
                                  out_sb[:])
        else:
            acc_all = ps_acc.tile([O + 1, CT, CW], F32, tag="acc")
            acc_tiles = [acc_all[:, c, :] for c in range(CT)]

            def _body(_iv=None):
                stg = _stage1()
                blk0 = 0
                for ci, nblk in enumerate(ADJ_CHUNKS):
                    adj_t = _adj_dma(blk0, nblk)
                    _stage2_blocks(stg, adj_t, range(blk0, blk0 + nblk))
                    blk0 += nblk
                _stage3()

        if hw_loop and repeat > 1:
            tc.For_i_unrolled(0, repeat, 1, _body, max_unroll=8)
        else:
            for _it in range(repeat):
                _body()

    nc.compile()
    return nc


_NC_CACHE = None


def prep_inputs(h, adj, W, b, a):
    """Host-side re-layout + derived small constants."""
    h = np.ascontiguousarray(h, dtype=np.float32)
    W = np.ascontiguousarray(W, dtype=np.float32)
    b = np.ascontiguousarray(b, dtype=np.float32).reshape(O)
    a = np.ascontiguousarray(a, dtype=np.float32)
    hT = np.swapaxes(h, -1, -2)
    # virtual node v = r*128 + p holds original node p*RB + r
    idx = (np.arange(128)[None, :] * RB + np.arange(RT)[:, None]).reshape(-1)
    hT = np.ascontiguousarray(hT[..., idx]).astype(np.float16)
    if ADJ_HOST_BF16:
        adjm = (np.asarray(adj) != 0).astype(ml_dtypes.bfloat16)
    else:
        adjm = (np.asarray(adj) != 0).astype(np.int8)

    a1, a2 = a[:O, 0], a[O:, 0]
    w1 = W @ a1                       # [F]
    w2 = W @ a2                       # [F]
    beta1 = float(b @ a1)
    beta2 = float(b @ a2)
    cst16 = np.zeros((128, C16W), np.float16)
    cst16[:, -3] = beta1
    cst16[:, -2] = ALPHA * beta1
    cst16[:, -1] = (1.0 - ALPHA) * beta2
    cst16[:, :128] = w2[:, None]      # w2 broadcast along free dim
    cst16[:, 128:128 + O] = W
    cst16[:, 128 + O] = w1
    bf = np.concatenate([b, [0.0]]).astype(np.float16)   # [b | 0] per block
    cst16[0, 128 + WCOL:128 + WCOL + PB * WCOL] = np.tile(bf, PB)
    # only row 0 is read by the e0 stationary; zero elsewhere already
    return hT, adjm, cst16


def make_in_maps(h, adj, W, b, a):
    hT, adjm, cst16 = prep_inputs(h, adj, W, b, a)
    return [
        {"hT": hT[i], "adjm": adjm[i], "cst16": cst16}
        for i in range(B)
    ]


def kernel(h, adj, W, b, a):
    global _NC_CACHE
    in_maps = make_in_maps(h, adj, W, b, a)
    if _NC_CACHE is None:
        _NC_CACHE = build_gat_kernel()
    nc = _NC_CACHE
    res = run_bass_kernel_spmd(nc, in_maps, core_ids=list(range(B)))
    out = np.stack([r["y"] for r in res.results], axis=0)
    return out
